# revision 19
# baseline (speedup 1.0000x reference)
"""Real spherical harmonics Y_lm (l<=8) on 8 TRN2 NeuronCores.

Data-parallel over the 1M points; per core 125k points padded to
128*977. Output is written in fp16, LEVEL-MAJOR: the slots for level
l are [Q0(l), (sin,cos)(l,1), ..., (sin,cos)(l,l)] with the diagonal
pair last, and columns are gamma-SCALED (Q = Y/gamma with gamma
chosen per chain so every l-recurrence reads

    Q(l) = x (.) Q(l-1) + c * Q(l-2)

with unit x-coefficient). The x-multiply for m0 + ALL chains at a
level is then ONE wide fp16 tensor_tensor whose in0 is x broadcast
via a stride-0 access pattern, and the c*Q(l-2) adds fuse into ONE
wide in-place tensor_tensor; the per-chain c-scale copies run on the
otherwise idle ACT engine (activation Copy with scale). The host
unshard un-permutes, descales, and converts to f32 (~1.4e-3 rel err
vs the 2e-2 gate).

All tensor ops run on DVE: the Pool engine shares SBUF ports with DVE
(concurrent Pool work measured to slow DVE ops ~2.8x), so Pool is
net-negative despite being idle.
"""

import math
import sys

sys.path.insert(0, "/opt/trn_rl_repo")

import numpy as np

import concourse.bass as bass
import concourse.mybir as mybir
from concourse.tile import TileContext
from concourse.bass_utils import run_bass_kernel_spmd

F32 = mybir.dt.float32
F16 = mybir.dt.float16
AF = mybir.ActivationFunctionType
OP = mybir.AluOpType

N_TOTAL = 1_000_000
NCORES = 8
PER = N_TOTAL // NCORES      # 125000 real points per core
P = 128                      # SBUF partitions
LPP = 977                    # points per partition (padded: 125056)
PADN = P * LPP
LMAX = 8
NCOL = (LMAX + 1) ** 2       # 81
FDS = [300, 300, 300, 77]    # free-dim chunk sizes (sum == LPP)

# ACT Sin LUT domain is [-pi, pi]; we feed t - pi, yielding -sin(t).
# The global -1 is folded into ctil(l,m) for m>=1.
PI_LO = float(np.nextafter(np.float32(math.pi), np.float32(0.0)))

# --- level-major slot order: base(l) = l^2; Q0(l) at l^2, sin(l,m) at
# l^2+2m-1, cos(l,m) at l^2+2m ---
ORDER = []
for _l in range(LMAX + 1):
    ORDER.append(_l * _l + _l)           # m=0 column
    for _m in range(1, _l + 1):
        ORDER.append(_l * _l + _l - _m)  # sin
        ORDER.append(_l * _l + _l + _m)  # cos
POSARR = np.empty(NCOL, np.int64)
for _i, _j in enumerate(ORDER):
    POSARR[_j] = _i


def _ctil():
    """ctil(l,m) * P_l^m(x) * ang(m, phi) = output column, with the
    reference's 1/sqrt(2) for m=0 folded in. m>=1 entries are negated
    to absorb the -sin from the range-reduced LUT trig."""
    c = {}
    for l in range(LMAX + 1):
        c[(l, 0)] = math.sqrt((2 * l + 1) / (4 * math.pi))
        for m in range(1, l + 1):
            c[(l, m)] = -((-1.0) ** m) * math.sqrt(2.0) * math.sqrt(
                (2 * l + 1) / (4 * math.pi)
                * math.factorial(l - m) / math.factorial(l + m)
            )
    return c


def _lrec_ab(l, m, C):
    """P~(l,m) = a*x*P~(l-1,m) + b*P~(l-2,m)."""
    alpha = (2 * l - 1) / (l - m)
    beta = -(l + m - 1) / (l - m)
    a = alpha * C[(l, m)] / C[(l - 1, m)]
    b = beta * C[(l, m)] / C[(l - 2, m)]
    return a, b


def _scales():
    """Per-column gamma (host descale), folded diag seeds, and the
    residual c coefficients for the scaled recurrences."""
    C = _ctil()
    kk = {1: 1.0, 2: 1.0}   # Dt(m) = kk_m * D_m ; Dt(m) = s2 (.) Dt(m-2)
    for m in range(3, LMAX + 1):
        Am = (2 * m - 1) * (2 * m - 3) * C[(m, m)] / C[(m - 2, m - 2)]
        kk[m] = kk[m - 2] / Am
    gamma = {}
    clm = {}
    for m in range(1, LMAX + 1):
        gamma[(m, m)] = 1.0 / kk[m]
        if m <= LMAX - 1:
            Em = (2 * m + 1) * C[(m + 1, m)] / C[(m, m)]
            gamma[(m + 1, m)] = Em * gamma[(m, m)]
            for l in range(m + 2, LMAX + 1):
                a, bb = _lrec_ab(l, m, C)
                gamma[(l, m)] = a * gamma[(l - 1, m)]
                clm[(l, m)] = bb * gamma[(l - 2, m)] / gamma[(l, m)]
    g0 = {0: C[(0, 0)], 1: C[(1, 0)]}
    c0 = {}
    for l in range(2, LMAX + 1):
        a, bb = _lrec_ab(l, 0, C)
        g0[l] = a * g0[l - 1]
        c0[l] = bb * g0[l - 2] / g0[l]
    gam = np.ones(NCOL, np.float32)
    for l in range(LMAX + 1):
        gam[l * l + l] = g0[l]
    for (l, m), g in gamma.items():
        gam[l * l + l - m] = g
        gam[l * l + l + m] = g
    return C, kk, clm, c0, gam


_C, _KK, _CLM, _C0, GAMMA = _scales()


def build_nc(fds=None):
    if fds is None:
        fds = FDS
    lpp = sum(fds)
    assert lpp == LPP
    C, kk, clm, c0 = _C, _KK, _CLM, _C0
    nc = bass.Bass()
    ct = nc.declare_dram_parameter("cos_theta", [PADN], F32, isOutput=False)
    ph = nc.declare_dram_parameter("phi", [PADN], F32, isOutput=False)
    out = nc.declare_dram_parameter("out", [PADN * NCOL], F16, isOutput=True)

    ctv = ct[:].rearrange("(p f) -> p f", p=P)
    phv = ph[:].rearrange("(p f) -> p f", p=P)
    outv = out[:].rearrange("(p f) -> p f", p=P)

    with TileContext(nc) as tc:
        with (
            tc.tile_pool(name="res", bufs=1) as res_pool,
            tc.tile_pool(name="work", bufs=2) as work_pool,
            tc.tile_pool(name="obuf", bufs=2) as o_pool,
        ):
            V = nc.vector
            A = nc.scalar

            xt = res_pool.tile([P, lpp], F32)
            pt = res_pool.tile([P, lpp], F32)
            cbias = res_pool.tile([P, 2], F32)
            nc.gpsimd.memset(cbias[:, 0:1], -PI_LO)
            nc.gpsimd.memset(cbias[:, 1:2], -PI_LO / 2)
            bias_negpi = cbias[:, 0:1]
            bias_neghalfpi = cbias[:, 1:2]

            # Full-width input loads, then x^2 and s = sqrt(1-x^2) once
            # at full width: sqrt lives in a different ACT LUT set than
            # sin/square/copy, so confining it to one op avoids two
            # 1.3us table swaps per chunk.
            nc.sync.dma_start(out=xt[:, :], in_=ctv[:, :])
            nc.sync.dma_start(out=pt[:, :], in_=phv[:, :])
            x2f = res_pool.tile([P, lpp], F32)
            sf = res_pool.tile([P, lpp], F32)
            A0 = nc.scalar
            A0.activation(x2f[:, :], xt[:, :], AF.Square)
            A0.activation(sf[:, :], x2f[:, :], AF.Sqrt, bias=1.0, scale=-1.0)

            off = 0
            for fd in fds:
                sl = slice(off, off + fd)
                obase = off * NCOL
                off += fd
                x = xt[:, sl]
                f = pt[:, sl]
                x2 = x2f[:, sl]
                s_ = sf[:, sl]

                # f32 work (24 fd-slices): 0 x2 | 1 s | 2,3 twoC1 pair |
                #  4,5 trig temp | 6..21 trig SIN|COS pairs (m at
                #  6+2(m-1)) | 22 b | 23 b2
                w = work_pool.tile([P, fd * 24], F32)
                # fp16 work (25 slices): 0,1 xx pair | 2..14 W arena
                #  (c-scaled adds, max width 13) | 15,16 s2 pair |
                #  17..24 diag ring (4 pairs)
                w6 = work_pool.tile([P, fd * 25], F16)

                def W(i):
                    return w[:, i * fd:(i + 1) * fd]

                def WF(i):
                    return w[:, i * fd:(i + 2) * fd]

                def H(i):
                    return w6[:, i * fd:(i + 1) * fd]

                def HF(i):
                    return w6[:, i * fd:(i + 2) * fd]

                XX = H(0)
                XXP = HF(0)

                def WA(k, n):   # W arena slots k..k+n
                    return w6[:, (2 + k) * fd:(2 + k + n) * fd]

                S2P = HF(15)

                def DT(m):
                    return HF(17 + 2 * (m & 3))

                def TRIGF(m):
                    return WF(6 + 2 * (m - 1))

                O = o_pool.tile([P, fd * NCOL], F16)

                def oblk(l, n):   # level-l block prefix, n slots
                    return O[:, l * l * fd:(l * l + n) * fd]

                def om0(l):
                    return oblk(l, 1)

                def opr(l, m):
                    b0 = l * l + 2 * m - 1
                    return O[:, b0 * fd:(b0 + 2) * fd]

                def bcast(ap, n):
                    return ap.rearrange("p (k f) -> p k f", k=1).broadcast_to(
                        [P, n, fd]
                    )

                def r3(ap):
                    return ap.rearrange("p (k f) -> p k f", f=fd)

                # ---- DVE self-starters (need only x, which lands
                # first): xx fp16 pair, m0 levels 0-1 ----
                V.tensor_scalar(H(0), x, 1.0, None, OP.mult)
                V.tensor_scalar(H(1), x, 1.0, None, OP.mult)
                V.tensor_scalar(om0(0), x, 0.0, 1.0, OP.mult, OP.add)
                V.tensor_scalar(om0(1), x, 1.0, None, OP.mult)
                V.tensor_tensor(om0(2), XX, om0(1), OP.mult)
                V.tensor_scalar(om0(2), om0(2), c0[2], None, OP.add)

                # ---- ACT seeds; trig group first (gates the chain) ----
                A.activation(W(6), f, AF.Sin, bias=bias_negpi)    # SIN1
                A.activation(W(22), f, AF.Sin, scale=0.5, bias=bias_neghalfpi)
                A.activation(W(23), W(22), AF.Square)             # b^2
                A.activation(W(2), W(23), AF.Copy, scale=4.0, bias=-2.0)
                A.activation(W(3), W(23), AF.Copy, scale=4.0, bias=-2.0)
                A.activation(W(7), W(23), AF.Copy, scale=-2.0, bias=1.0)
                c11, c22 = C[(1, 1)], C[(2, 2)]
                d1 = -c11 * kk[1]
                d2 = 3.0 * c22 * kk[2]
                A.activation(H(19), s_, AF.Copy, scale=d1)   # Dt1 pair
                A.activation(H(20), s_, AF.Copy, scale=d1)
                A.activation(H(21), x2, AF.Copy, scale=-d2, bias=d2)
                A.activation(H(22), x2, AF.Copy, scale=-d2, bias=d2)
                A.activation(H(15), x2, AF.Copy, scale=-1.0, bias=1.0)  # s2
                A.activation(H(16), x2, AF.Copy, scale=-1.0, bias=1.0)

                # ---- trig Chebyshev (f32 pairs) ----
                V.tensor_tensor(TRIGF(2), WF(2), TRIGF(1), OP.mult)
                V.tensor_scalar(W(9), W(9), 1.0, None, OP.add)  # COS2 += 1
                for m in range(3, 9):
                    V.tensor_tensor(WF(4), WF(2), TRIGF(m - 1), OP.mult)
                    V.tensor_tensor(TRIGF(m), WF(4), TRIGF(m - 2), OP.subtract)

                # ---- levels 1..2 column seeds ----
                V.tensor_tensor(opr(1, 1), DT(1), TRIGF(1), OP.mult)
                V.tensor_tensor(opr(2, 1), XXP, opr(1, 1), OP.mult)
                V.tensor_tensor(opr(2, 2), DT(2), TRIGF(2), OP.mult)

                # ---- levels 3..8: wide x-multiply over [m0 + lrec
                # chains], ACT c-scale preps, wide in-place add, then
                # first-l, diag, colmult ----
                for l in range(3, 9):
                    wd = 1 + 2 * (l - 2)   # m0 + chains m=1..l-2
                    V.tensor_tensor(
                        r3(oblk(l, wd)), bcast(XX, wd), r3(oblk(l - 1, wd)),
                        OP.mult,
                    )
                    # c-scale preps on ACT: W arena mirrors the FULL
                    # level-(l-2) block (its last pair is chain m=l-2's
                    # Q(l-2), i.e. the diagonal pair)
                    A.activation(WA(0, 1), om0(l - 2), AF.Copy, scale=c0[l])
                    for m in range(1, l - 1):
                        A.activation(
                            WA(2 * m - 1, 2), opr(l - 2, m), AF.Copy,
                            scale=clm[(l, m)],
                        )
                    V.tensor_tensor(
                        oblk(l, wd), oblk(l, wd), WA(0, wd), OP.add
                    )
                    V.tensor_tensor(opr(l, l - 1), XXP, opr(l - 1, l - 1), OP.mult)
                    V.tensor_tensor(DT(l), S2P, DT(l - 2), OP.mult)
                    V.tensor_tensor(opr(l, l), DT(l), TRIGF(l), OP.mult)
                    # stream finished level groups out early; the final
                    # DMA after the last compute is then only level 8
                    if l in (5, 7, 8):
                        s0 = {5: 0, 7: 36, 8: 64}[l] * fd
                        s1 = (l + 1) * (l + 1) * fd
                        nc.sync.dma_start(
                            out=outv[:, obase + s0:obase + s1],
                            in_=O[:, s0:s1],
                        )
    _legalize_waits(nc)
    return nc


def _legalize_waits(nc):
    """TPB compute ISA structs encode a single sync-wait slot; Tile can
    emit 2+ waits on one instruction (walrus then fails with 'Too many
    sync wait commands'). Hoist all but one wait onto NoOps in front."""
    f = nc.m.functions[0]
    for b in f.blocks:
        insts = b.instructions
        idx = 0
        while idx < len(insts):
            i = insts[idx]
            si = i.sync_info
            if si is not None and len(si.on_wait) > 1:
                waits = list(si.on_wait)
                for wextra in waits[:-1]:
                    nop = mybir.InstEventSemaphore(
                        name=nc.get_next_instruction_name(), ins=[], outs=[]
                    )
                    nop.engine = i.engine
                    nop.sync_info = mybir.SyncInfo(
                        on_wait=[wextra], on_update=[]
                    )
                    nc.register_instruction(nop)
                    insts.insert(idx, nop)
                    idx += 1
                si.on_wait = [waits[-1]]
            idx += 1


_NC_CACHE = None


def _get_nc():
    global _NC_CACHE
    if _NC_CACHE is None:
        _NC_CACHE = build_nc()
    return _NC_CACHE


def _run(cos_theta, phi, trace=False, **kw):
    cos_theta = np.ascontiguousarray(np.asarray(cos_theta), dtype=np.float32)
    phi = np.ascontiguousarray(np.asarray(phi), dtype=np.float32)
    assert cos_theta.shape == (N_TOTAL,) and phi.shape == (N_TOTAL,)
    in_maps = []
    for i in range(NCORES):
        c = np.zeros(PADN, np.float32)
        p_ = np.zeros(PADN, np.float32)
        c[:PER] = cos_theta[i * PER:(i + 1) * PER]
        p_[:PER] = phi[i * PER:(i + 1) * PER]
        in_maps.append({"cos_theta": c, "phi": p_})
    res = run_bass_kernel_spmd(
        _get_nc(), in_maps, core_ids=list(range(NCORES)), trace=trace, **kw
    )
    gscale = GAMMA[None, :, None]  # descale in original-column order
    outs = []
    for r in res.results:
        o = np.asarray(r["out"]).reshape(P, LPP * NCOL)  # fp16, slot-major
        full = np.empty((P, LPP, NCOL), np.float32)
        offp = 0
        for fd in FDS:
            blk = o[:, offp * NCOL:(offp + fd) * NCOL].reshape(P, NCOL, fd)
            full[:, offp:offp + fd, :] = (
                blk[:, POSARR, :] * gscale
            ).transpose(0, 2, 1)
            offp += fd
        outs.append(full.reshape(PADN, NCOL)[:PER])
    return np.concatenate(outs, axis=0), res


def kernel(cos_theta, phi):
    out, _ = _run(cos_theta, phi)
    return out
